# revision 1
# baseline (speedup 1.0000x reference)
"""Self-contained Trainium2 Bass kernel for the EdgeNetwork GNN problem.

kernel(**inputs) takes the FULL unsharded inputs and returns the FULL
[100000, 32] output.

Strategy: shard by DESTINATION node range across 8 cores (no collectives
needed). Host routes each edge to the core owning its dst, sorts by dst,
and packs edges into 512-edge chunks such that no dst-run crosses a
chunk boundary. Per chunk the device:
  - indirect-DMA gathers neighbor features x = node_attr[src]
  - builds the Khatri-Rao expansion Z[e,(k,j)] = ea[e,k]*x[e,j] with a
    single broadcast-AP tensor_tensor multiply per 128-edge tile
  - PE-transposes Z into contraction-major layout ZT
  - computes msg^T = sum_g B_g^T @ ZT_g on the tensor engine (PSUM accum)
  - segment-sums sorted dst-runs with a masked tensor_tensor_scan
  - PE-transposes run totals back to row layout and indirect-DMA
    scatters them (sentinel indices are bounds-check-skipped)
"""

import os
import sys
from contextlib import ExitStack

import numpy as np

for _p in ("/opt/trn_rl_repo", "/root/.axon_site/_ro/trn_rl_repo"):
    if os.path.isdir(_p) and _p not in sys.path:
        sys.path.insert(0, _p)

import concourse.mybir as mybir
import concourse.tile as tile
from concourse import bacc
from concourse.bass import IndirectOffsetOnAxis
from concourse.bass_utils import run_bass_kernel_spmd
from concourse.masks import make_identity

N_NODES = 100000
D = 32
KE = 16
NCORES = 8
NPC = N_NODES // NCORES
CHUNK = 512
SUPER = 4096
SENTINEL = 16384  # > NPC-1 and small enough that idx*row_stride fits int32

F32 = mybir.dt.float32
I32 = mybir.dt.int32


# ---------------------------------------------------------------- host prep

def _pack_core_edges(dst_sorted_idx, dst_local):
    n = len(dst_sorted_idx)
    order, mask, run_end_pos = [], [], []
    i = 0
    while i < n:
        j = i
        while j < n and dst_local[j] == dst_local[i]:
            j += 1
        run_len = j - i
        assert run_len <= CHUNK
        used = len(order) % CHUNK
        if used + run_len > CHUNK:
            pad = CHUNK - used
            order.extend([-1] * pad)
            mask.extend([1.0] * pad)
        for r in range(i, j):
            order.append(dst_sorted_idx[r])
            mask.append(0.0 if r == i else 1.0)
        run_end_pos.append(len(order) - 1)
        i = j
    order = np.asarray(order, dtype=np.int64)
    mask = np.asarray(mask, dtype=np.float32)
    is_end = np.zeros(len(order), dtype=bool)
    if run_end_pos:
        is_end[np.asarray(run_end_pos, dtype=np.int64)] = True
    return order, mask, is_end


def _prepare(node_attr, edge_attr, pair_indices, kernel, bias):
    dst = np.asarray(pair_indices[:, 0], dtype=np.int64)
    src = np.asarray(pair_indices[:, 1], dtype=np.int64)
    ea = np.asarray(edge_attr, dtype=np.float32)
    kern = np.asarray(kernel, dtype=np.float32)
    bias = np.asarray(bias, dtype=np.float32)

    use_bias = bool(np.any(bias != 0.0))
    if use_bias:
        KP = KE + 1
        kern_full = np.concatenate([kern, bias[None, :]], axis=0)
    else:
        KP = KE
        kern_full = kern
    KG = (KP + 3) // 4
    KPAD = KG * 4

    B = np.zeros((KPAD * D, D), dtype=np.float32)
    Bk = kern_full.reshape(KP, D, D).transpose(0, 2, 1)
    B[: KP * D] = Bk.reshape(KP * D, D)

    per_core_raw = []
    max_len = 0
    for c in range(NCORES):
        lo, hi = c * NPC, (c + 1) * NPC
        sel = np.nonzero((dst >= lo) & (dst < hi))[0]
        d_loc_unsorted = dst[sel] - lo
        s_ord = np.argsort(d_loc_unsorted, kind="stable")
        order, mask, is_end = _pack_core_edges(sel[s_ord],
                                               d_loc_unsorted[s_ord])
        per_core_raw.append((order, mask, is_end))
        max_len = max(max_len, len(order))

    Epad = ((max_len + SUPER - 1) // SUPER) * SUPER
    NSUP = Epad // SUPER

    per_core = []
    node_attr_f = np.ascontiguousarray(node_attr, dtype=np.float32)
    for c in range(NCORES):
        order, mask, is_end = per_core_raw[c]
        n = len(order)
        pad = Epad - n
        order = np.concatenate([order, np.full(pad, -1, np.int64)])
        mask = np.concatenate([mask, np.ones(pad, np.float32)])
        is_end = np.concatenate([is_end, np.zeros(pad, bool)])

        real = order >= 0
        oe = np.where(real, order, 0)

        eaP = np.zeros((Epad, KPAD), dtype=np.float32)
        eaP[real, :KE] = ea[oe[real]]
        if use_bias:
            eaP[real, KE] = 1.0
        srcP = np.where(real, src[oe], 0).astype(np.int32)
        dstP = (dst[oe] - c * NPC).astype(np.int32)
        sidxP = np.where(is_end, dstP, SENTINEL).astype(np.int32)

        def swz(a):
            a = a.reshape(NSUP, 8, 4, 128, *a.shape[1:])
            return np.ascontiguousarray(np.moveaxis(a, 3, 1))

        per_core.append(dict(
            ea_sw=swz(eaP).reshape(NSUP, 128, 32 * KPAD),
            src_sw=swz(srcP).reshape(NSUP, 128, 32),
            sidx_sw=swz(sidxP).reshape(NSUP, 128, 32),
            maskT=np.ascontiguousarray(
                np.broadcast_to(mask[None, :], (D, Epad))),
            node_attr=node_attr_f,
            B=B,
        ))
    meta = dict(Epad=Epad, NSUP=NSUP, KG=KG, KPAD=KPAD)
    return per_core, meta


# ------------------------------------------------------------- bass program

def _build(NSUP, KPAD, KG):
    nc = bacc.Bacc("TRN2", target_bir_lowering=False, debug=False)

    ea_d = nc.dram_tensor("ea_sw", [NSUP, 128, 32 * KPAD], F32,
                          kind="ExternalInput").ap()
    src_d = nc.dram_tensor("src_sw", [NSUP, 128, 32], I32,
                           kind="ExternalInput").ap()
    sidx_d = nc.dram_tensor("sidx_sw", [NSUP, 128, 32], I32,
                            kind="ExternalInput").ap()
    mask_d = nc.dram_tensor("maskT", [D, NSUP * SUPER], F32,
                            kind="ExternalInput").ap()
    node_d = nc.dram_tensor("node_attr", [N_NODES, D], F32,
                            kind="ExternalInput").ap()
    b_d = nc.dram_tensor("B", [KG * 128, D], F32, kind="ExternalInput").ap()
    out_d = nc.dram_tensor("out", [NPC, D], F32, kind="ExternalOutput").ap()

    with tile.TileContext(nc) as tc, ExitStack() as ctx:
        const_pool = ctx.enter_context(tc.tile_pool(name="const", bufs=1))
        sup_pool = ctx.enter_context(tc.tile_pool(name="sup", bufs=2))
        x_pool = ctx.enter_context(tc.tile_pool(name="x", bufs=8))
        z_pool = ctx.enter_context(tc.tile_pool(name="z", bufs=8))
        zt_pool = ctx.enter_context(tc.tile_pool(name="zt", bufs=3))
        sc_pool = ctx.enter_context(tc.tile_pool(name="sc", bufs=2))
        ot_pool = ctx.enter_context(tc.tile_pool(name="ot", bufs=8))
        pz_pool = ctx.enter_context(
            tc.tile_pool(name="pz", bufs=3, space="PSUM"))
        pm_pool = ctx.enter_context(
            tc.tile_pool(name="pm", bufs=2, space="PSUM"))
        po_pool = ctx.enter_context(
            tc.tile_pool(name="po", bufs=2, space="PSUM"))

        ident = const_pool.tile([128, 128], F32, tag="ident")
        make_identity(nc, ident[:])
        b_sb = const_pool.tile([128, KG * D], F32, tag="b")
        for g in range(KG):
            nc.sync.dma_start(b_sb[:, g * D:(g + 1) * D],
                              b_d[g * 128:(g + 1) * 128, :])

        for s in range(NSUP):
            ea_sb = sup_pool.tile([128, 32 * KPAD], F32, tag="ea")
            nc.sync.dma_start(ea_sb[:], ea_d[s])
            src_sb = sup_pool.tile([128, 32], I32, tag="src")
            nc.sync.dma_start(src_sb[:], src_d[s])
            sidx_sb = sup_pool.tile([128, 32], I32, tag="sidx")
            nc.sync.dma_start(sidx_sb[:], sidx_d[s])
            mask_sb = sup_pool.tile([D, SUPER], F32, tag="mask")
            nc.sync.dma_start(mask_sb[:],
                              mask_d[:, s * SUPER:(s + 1) * SUPER])

            for q in range(8):
                z_tiles = []
                for t in range(4):
                    qt = q * 4 + t
                    x_t = x_pool.tile([128, D], F32, tag="x")
                    nc.gpsimd.indirect_dma_start(
                        out=x_t[:], out_offset=None, in_=node_d[:],
                        in_offset=IndirectOffsetOnAxis(
                            ap=src_sb[:, qt:qt + 1], axis=0))
                    z_t = z_pool.tile([128, KPAD * D], F32, tag="z")
                    x_b = x_t[:].rearrange("p (o j) -> p o j", o=1) \
                        .to_broadcast([128, KPAD, D])
                    ea_b = ea_sb[:, qt * KPAD:(qt + 1) * KPAD] \
                        .rearrange("p (k o) -> p k o", o=1) \
                        .to_broadcast([128, KPAD, D])
                    nc.vector.tensor_tensor(
                        out=z_t[:].rearrange("p (k j) -> p k j", j=D),
                        in0=x_b, in1=ea_b, op=mybir.AluOpType.mult)
                    z_tiles.append(z_t)

                zt_sb = zt_pool.tile([128, KG * CHUNK], F32, tag="zt")
                for g in range(KG):
                    pz = pz_pool.tile([128, CHUNK], F32, tag="pz")
                    for t in range(4):
                        nc.tensor.transpose(
                            out=pz[:, t * 128:(t + 1) * 128],
                            in_=z_tiles[t][:, g * 128:(g + 1) * 128],
                            identity=ident[:])
                    if g % 2 == 0:
                        nc.scalar.copy(
                            out=zt_sb[:, g * CHUNK:(g + 1) * CHUNK],
                            in_=pz[:])
                    else:
                        nc.vector.tensor_copy(
                            out=zt_sb[:, g * CHUNK:(g + 1) * CHUNK],
                            in_=pz[:])

                pm = pm_pool.tile([D, CHUNK], F32, tag="pm")
                for g in range(KG):
                    nc.tensor.matmul(
                        out=pm[:], lhsT=b_sb[:, g * D:(g + 1) * D],
                        rhs=zt_sb[:, g * CHUNK:(g + 1) * CHUNK],
                        start=(g == 0), stop=(g == KG - 1))

                scano = sc_pool.tile([D, CHUNK], F32, tag="sc")
                nc.vector.tensor_tensor_scan(
                    out=scano[:],
                    data0=mask_sb[:, q * CHUNK:(q + 1) * CHUNK],
                    data1=pm[:], initial=0.0,
                    op0=mybir.AluOpType.mult, op1=mybir.AluOpType.add)

                po = po_pool.tile([128, 4 * D], F32, tag="po")
                for t in range(4):
                    nc.tensor.transpose(
                        out=po[:, t * D:(t + 1) * D],
                        in_=scano[:, t * 128:(t + 1) * 128],
                        identity=ident[:D, :D])
                ot = ot_pool.tile([128, 4 * D], F32, tag="ot")
                if q % 2 == 0:
                    nc.scalar.copy(out=ot[:], in_=po[:])
                else:
                    nc.vector.tensor_copy(out=ot[:], in_=po[:])
                for t in range(4):
                    qt = q * 4 + t
                    nc.gpsimd.indirect_dma_start(
                        out=out_d[:],
                        out_offset=IndirectOffsetOnAxis(
                            ap=sidx_sb[:, qt:qt + 1], axis=0),
                        in_=ot[:, t * D:(t + 1) * D], in_offset=None,
                        bounds_check=NPC - 1, oob_is_err=False)

    nc.compile()
    return nc


_CACHE = {}


def kernel(node_attr, edge_attr, pair_indices, kernel, bias):
    per_core, meta = _prepare(node_attr, edge_attr, pair_indices,
                              kernel, bias)
    key = (meta["NSUP"], meta["KPAD"], meta["KG"])
    if key not in _CACHE:
        _CACHE[key] = _build(*key)
    nc = _CACHE[key]
    res = run_bass_kernel_spmd(nc, per_core, list(range(NCORES)))
    out = np.concatenate([res.results[c]["out"] for c in range(NCORES)],
                         axis=0)
    return np.ascontiguousarray(out, dtype=np.float32)



# revision 2
# speedup vs baseline: 1.9565x; 1.9565x over previous
"""Self-contained Trainium2 Bass kernel for the EdgeNetwork GNN problem.

kernel(**inputs) takes the FULL unsharded inputs and returns the FULL
[100000, 32] output.

Strategy: shard by DESTINATION node range across 8 cores (no collectives).
Host routes each edge to the core owning its dst, sorts by dst, and packs
edges into 512-edge chunks (max 128 distinct dst "runs" per chunk, no run
crosses a chunk boundary).  Per chunk the device computes

    U^T[(k,j), n] = sum_e S[e, n] * ea[e, k] * x[e, j]      (PE matmuls)
    out[n, i]     = sum_{k,j} U^T[(k,j), n] * B[(k,j), i]   (PE matmuls)

where S[e, n] = 1 iff edge e belongs to the chunk's n-th dst run.  S is
materialised by an indirect-DMA gather of identity-matrix rows (row 128 is
all zeros, used by padding edges), so no vector-engine work is needed for
the segment sum at all.  The only DVE work is the Khatri-Rao product
Z[e, (k,j)] = ea[e,k] * x[e,j], one broadcast tensor_tensor per chunk.
Gathers/scatters are batched per 4096-edge superstep to amortise SWDGE
descriptor generation.  All PE traffic is fp16 (1 cycle/row); accumulation
happens in fp32 PSUM.
"""

import os
import sys
from contextlib import ExitStack

import numpy as np

for _p in ("/opt/trn_rl_repo", "/root/.axon_site/_ro/trn_rl_repo"):
    if os.path.isdir(_p) and _p not in sys.path:
        sys.path.insert(0, _p)

import concourse.mybir as mybir
import concourse.tile as tile
from concourse import bacc
from concourse.bass import IndirectOffsetOnAxis
from concourse.bass_utils import run_bass_kernel_spmd

N_NODES = 100000
D = 32
KE = 16
NCORES = 8
NPC = N_NODES // NCORES
CHUNK = 512          # edges per chunk (4 tiles of 128)
NRUNS = 128          # max dst runs per chunk (S-matrix columns)
SUPER = 4096         # edges per superstep (8 chunks)
SENTINEL = 16384     # scatter offset for unused run slots (> NPC-1)

F32 = mybir.dt.float32
F16 = mybir.dt.float16
I32 = mybir.dt.int32


# ---------------------------------------------------------------- host prep

def _pack_core_edges(dst_sorted_idx, dst_local):
    """Pack dst-sorted edges into chunks of <=CHUNK edges and <=NRUNS runs.

    Returns (order, slot, sidx):
      order [n_chunks*CHUNK] int64: edge id per packed position (-1 = pad)
      slot  [n_chunks*CHUNK] int32: run index within chunk (NRUNS = pad)
      sidx  [n_chunks, NRUNS] int32: local dst node per run (SENTINEL = unused)
    """
    n = len(dst_sorted_idx)
    # run boundaries in the sorted edge list
    starts = np.flatnonzero(np.diff(dst_local, prepend=-1))
    lengths = np.diff(starts, append=n)
    assert lengths.max(initial=0) <= CHUNK, "single dst exceeds chunk capacity"
    run_dst = dst_local[starts]

    order_chunks, slot_chunks, sidx_chunks = [], [], []
    cur_order, cur_slot, cur_sidx = [], [], []

    def flush():
        pad = CHUNK - len(cur_order)
        order_chunks.append(np.concatenate(
            [np.asarray(cur_order, np.int64), np.full(pad, -1, np.int64)]))
        s = np.concatenate(
            [np.asarray(cur_slot, np.int32), np.full(pad, NRUNS, np.int32)])
        slot_chunks.append(s)
        si = np.full(NRUNS, SENTINEL, np.int32)
        si[:len(cur_sidx)] = cur_sidx
        sidx_chunks.append(si)
        cur_order.clear(); cur_slot.clear(); cur_sidx.clear()

    for r in range(len(starts)):
        ln = int(lengths[r])
        if len(cur_order) + ln > CHUNK or len(cur_sidx) >= NRUNS:
            flush()
        cur_slot.extend([len(cur_sidx)] * ln)
        cur_order.extend(dst_sorted_idx[starts[r]:starts[r] + ln])
        cur_sidx.append(int(run_dst[r]))
    if cur_order or cur_sidx:
        flush()

    if not order_chunks:
        order_chunks.append(np.full(CHUNK, -1, np.int64))
        slot_chunks.append(np.full(CHUNK, NRUNS, np.int32))
        sidx_chunks.append(np.full(NRUNS, SENTINEL, np.int32))

    return (np.concatenate(order_chunks), np.concatenate(slot_chunks),
            np.stack(sidx_chunks))


def _prepare(node_attr, edge_attr, pair_indices, kernel, bias):
    dst = np.asarray(pair_indices[:, 0], dtype=np.int64)
    src = np.asarray(pair_indices[:, 1], dtype=np.int64)
    ea = np.asarray(edge_attr, dtype=np.float32)
    kern = np.asarray(kernel, dtype=np.float32)
    bias = np.asarray(bias, dtype=np.float32)

    use_bias = bool(np.any(bias != 0.0))
    if use_bias:
        KP = KE + 1
        kern_full = np.concatenate([kern, bias[None, :]], axis=0)
    else:
        KP = KE
        kern_full = kern
    KG = (KP + 3) // 4
    KPAD = KG * 4

    # B[(k,j), i] = kern[k, i*D + j], zero-padded to KPAD k's
    B = np.zeros((KPAD * D, D), dtype=np.float32)
    Bk = kern_full.reshape(KP, D, D).transpose(0, 2, 1)   # [KP, j, i]
    B[: KP * D] = Bk.reshape(KP * D, D)

    # identity gather table: row s = e_s for s < NRUNS, row NRUNS = zeros
    IDENT = np.zeros((NRUNS + 1, NRUNS), dtype=np.float16)
    IDENT[:NRUNS, :NRUNS] = np.eye(NRUNS, dtype=np.float16)

    per_core_raw = []
    max_chunks = 0
    for c in range(NCORES):
        lo, hi = c * NPC, (c + 1) * NPC
        sel = np.nonzero((dst >= lo) & (dst < hi))[0]
        d_loc_unsorted = dst[sel] - lo
        s_ord = np.argsort(d_loc_unsorted, kind="stable")
        order, slot, sidx = _pack_core_edges(sel[s_ord],
                                             d_loc_unsorted[s_ord])
        per_core_raw.append((order, slot, sidx))
        max_chunks = max(max_chunks, len(sidx))

    NSUP = (max_chunks + 7) // 8
    NCH = NSUP * 8
    Epad = NCH * CHUNK

    def swz(a):
        # [NSUP*8*4*128, ...] -> [NSUP, 128, 8*4, ...] (col = q*4 + t)
        a = a.reshape(NSUP, 8, 4, 128, *a.shape[1:])
        return np.ascontiguousarray(np.moveaxis(a, 3, 1))

    per_core = []
    node_f16 = np.ascontiguousarray(node_attr, dtype=np.float16)
    for c in range(NCORES):
        order, slot, sidx = per_core_raw[c]
        nch = len(sidx)
        order = np.concatenate([order, np.full((NCH - nch) * CHUNK, -1,
                                               np.int64)])
        slot = np.concatenate([slot, np.full((NCH - nch) * CHUNK, NRUNS,
                                             np.int32)])
        sidx = np.concatenate([sidx, np.full((NCH - nch, NRUNS), SENTINEL,
                                             np.int32)])

        real = order >= 0
        oe = np.where(real, order, 0)

        eaP = np.zeros((Epad, KPAD), dtype=np.float16)
        eaP[real, :KE] = ea[oe[real]].astype(np.float16)
        if use_bias:
            eaP[real, KE] = 1.0
        srcP = np.where(real, src[oe], 0).astype(np.int32)

        # sidx: [NCH, NRUNS] -> [NSUP, 8, 128] -> [NSUP, 128, 8]
        sidx_sw = np.ascontiguousarray(
            sidx.reshape(NSUP, 8, NRUNS).transpose(0, 2, 1))

        per_core.append(dict(
            ea_sw=swz(eaP).reshape(NSUP, 128, 32 * KPAD),
            src_sw=swz(srcP).reshape(NSUP, 128, 32),
            slot_sw=swz(slot).reshape(NSUP, 128, 32),
            sidx_sw=sidx_sw,
            node_attr=node_f16,
            B=B.astype(np.float16),
            IDENT=IDENT,
        ))
    meta = dict(Epad=Epad, NSUP=NSUP, KG=KG, KPAD=KPAD)
    return per_core, meta


# ------------------------------------------------------------- bass program

def _build(NSUP, KPAD, KG):
    nc = bacc.Bacc("TRN2", target_bir_lowering=False, debug=False)

    KJ = KPAD * D            # Khatri-Rao width (kj columns), KG blocks of 128
    ea_d = nc.dram_tensor("ea_sw", [NSUP, 128, 32 * KPAD], F16,
                          kind="ExternalInput").ap()
    src_d = nc.dram_tensor("src_sw", [NSUP, 128, 32], I32,
                           kind="ExternalInput").ap()
    slot_d = nc.dram_tensor("slot_sw", [NSUP, 128, 32], I32,
                            kind="ExternalInput").ap()
    sidx_d = nc.dram_tensor("sidx_sw", [NSUP, 128, 8], I32,
                            kind="ExternalInput").ap()
    node_d = nc.dram_tensor("node_attr", [N_NODES, D], F16,
                            kind="ExternalInput").ap()
    b_d = nc.dram_tensor("B", [KJ, D], F16, kind="ExternalInput").ap()
    id_d = nc.dram_tensor("IDENT", [NRUNS + 1, NRUNS], F16,
                          kind="ExternalInput").ap()
    out_d = nc.dram_tensor("out", [NPC, D], F32, kind="ExternalOutput").ap()

    with tile.TileContext(nc) as tc, ExitStack() as ctx:
        const_pool = ctx.enter_context(tc.tile_pool(name="const", bufs=1))
        sup_pool = ctx.enter_context(tc.tile_pool(name="sup", bufs=2))
        z_pool = ctx.enter_context(tc.tile_pool(name="z", bufs=3))
        ut_pool = ctx.enter_context(tc.tile_pool(name="ut", bufs=3))
        ot_pool = ctx.enter_context(tc.tile_pool(name="ot", bufs=2))
        put_pool = ctx.enter_context(
            tc.tile_pool(name="put", bufs=3, space="PSUM"))
        po_pool = ctx.enter_context(
            tc.tile_pool(name="po", bufs=2, space="PSUM"))

        b_sb = const_pool.tile([128, KG * D], F16, tag="b")
        for g in range(KG):
            nc.sync.dma_start(b_sb[:, g * D:(g + 1) * D],
                              b_d[g * 128:(g + 1) * 128, :])

        for s in range(NSUP):
            ea_sb = sup_pool.tile([128, 32 * KPAD], F16, tag="ea")
            nc.sync.dma_start(ea_sb[:], ea_d[s])
            src_sb = sup_pool.tile([128, 32], I32, tag="src")
            nc.sync.dma_start(src_sb[:], src_d[s])
            slot_sb = sup_pool.tile([128, 32], I32, tag="slot")
            nc.sync.dma_start(slot_sb[:], slot_d[s])
            sidx_sb = sup_pool.tile([128, 8], I32, tag="sidx")
            nc.sync.dma_start(sidx_sb[:], sidx_d[s])

            # batched gathers: x rows and S (identity) rows for all 32 tiles
            x_sb = sup_pool.tile([128, 32 * D], F16, tag="x")
            nc.gpsimd.indirect_dma_start(
                out=x_sb[:], out_offset=None, in_=node_d[:],
                in_offset=IndirectOffsetOnAxis(ap=src_sb[:], axis=0))
            s_sb = sup_pool.tile([128, 32 * NRUNS], F16, tag="s")
            nc.gpsimd.indirect_dma_start(
                out=s_sb[:], out_offset=None, in_=id_d[:],
                in_offset=IndirectOffsetOnAxis(ap=slot_sb[:], axis=0))

            ot = ot_pool.tile([128, 8 * D], F32, tag="ot")

            for q in range(8):
                # Z[e, (t,k,j)] = ea[e, (t,k)] * x[e, (t,j)] for whole chunk
                z_t = z_pool.tile([128, 4 * KJ], F16, tag="z")
                x_b = x_sb[:, q * 4 * D:(q + 1) * 4 * D] \
                    .rearrange("p (t o j) -> p t o j", t=4, o=1) \
                    .to_broadcast([128, 4, KPAD, D])
                ea_b = ea_sb[:, q * 4 * KPAD:(q + 1) * 4 * KPAD] \
                    .rearrange("p (t k o) -> p t k o", t=4, o=1) \
                    .to_broadcast([128, 4, KPAD, D])
                nc.vector.tensor_tensor(
                    out=z_t[:].rearrange("p (t k j) -> p t k j", t=4, j=D),
                    in0=x_b, in1=ea_b, op=mybir.AluOpType.mult)

                # UT[(kj), n] += Z[e, kj]^T @ S[e, n]  (contract edges)
                ut_ps = put_pool.tile([128, KG * NRUNS], F32, tag="utp")
                for g in range(KG):
                    for t in range(4):
                        nc.tensor.matmul(
                            out=ut_ps[:, g * NRUNS:(g + 1) * NRUNS],
                            lhsT=z_t[:, t * KJ + g * 128:t * KJ + (g + 1) * 128],
                            rhs=s_sb[:, (q * 4 + t) * NRUNS:
                                     (q * 4 + t + 1) * NRUNS],
                            start=(t == 0), stop=(t == 3))

                ut_sb = ut_pool.tile([128, KG * NRUNS], F16, tag="uts")
                nc.scalar.copy(out=ut_sb[:], in_=ut_ps[:])

                # out[n, i] = sum_g UT_g[kj, n]^T @ B_g[kj, i]
                po = po_pool.tile([128, D], F32, tag="po")
                for g in range(KG):
                    nc.tensor.matmul(
                        out=po[:],
                        lhsT=ut_sb[:, g * NRUNS:(g + 1) * NRUNS],
                        rhs=b_sb[:, g * D:(g + 1) * D],
                        start=(g == 0), stop=(g == KG - 1))
                nc.scalar.copy(out=ot[:, q * D:(q + 1) * D], in_=po[:])

            # batched scatter: row (p, q) -> out_d[sidx[p, q]]
            nc.gpsimd.indirect_dma_start(
                out=out_d[:],
                out_offset=IndirectOffsetOnAxis(ap=sidx_sb[:], axis=0),
                in_=ot[:], in_offset=None,
                bounds_check=NPC - 1, oob_is_err=False)

    nc.compile()
    return nc


_CACHE = {}


def kernel(node_attr, edge_attr, pair_indices, kernel, bias):
    per_core, meta = _prepare(node_attr, edge_attr, pair_indices,
                              kernel, bias)
    key = (meta["NSUP"], meta["KPAD"], meta["KG"])
    if key not in _CACHE:
        _CACHE[key] = _build(*key)
    nc = _CACHE[key]
    res = run_bass_kernel_spmd(nc, per_core, list(range(NCORES)))
    out = np.concatenate([res.results[c]["out"] for c in range(NCORES)],
                         axis=0)
    return np.ascontiguousarray(out, dtype=np.float32)
